# revision 17
# baseline (speedup 1.0000x reference)
"""Pairwise L2 distance kernel: x [4096,768], anchors [100,64,768] -> [4096,100,64].

Distributed over 8 TRN2 NeuronCores as a 2x4 grid: batch (4096) split in 2,
anchor index (6400) split in 4.  Each core computes a [2048,1600] output block
as sqrt(x2[b] + a2[j] - 2*x@A^T).

The x@A^T matmul runs in fp8e4m3 with DoubleRow (K=256 per pass, fp32 PSUM
accumulate); norms are computed on device (x2 from a bf16 copy of x via
ACT Square+accum, a2 via DVE square + all-(-0.5) ones-matmul broadcast).
Host does layout transforms only (transpose, dtype cast, partition packing).
"""

import sys

import numpy as np

for _p in ("/opt/trn_rl_repo", "/root/.axon_site/_ro/trn_rl_repo"):
    if _p not in sys.path:
        sys.path.append(_p)

import ml_dtypes

import concourse.bass as bass
import concourse.tile as tile
from concourse import bacc, mybir
from concourse.bass import ts
from concourse.bass_utils import run_bass_kernel_spmd

B, C, A, E = 4096, 100, 64, 768
J = C * A                 # 6400 flattened anchors
RB, RJ = 2, 4             # batch groups x anchor groups = 8 cores
MB = B // RB              # 2048 batch rows per core
NJ = J // RJ              # 1600 anchor cols per core
KT = E // 128             # 6 contraction tiles of 128
K2 = KT // 2              # 3 DoubleRow k-pair passes
MT = MB // 128            # 16 m-tiles per core
XO_Q = 8                  # xo arrives in 8 DMA slices
N_CHUNKS = [(0, 512), (512, 512), (1024, 512), (1536, 64)]
PSW = 2048                # psum tile width (4 banks), holds all chunks

FP8 = mybir.dt.float8e4
BF16 = mybir.dt.bfloat16
F32 = mybir.dt.float32
NP_FP8 = ml_dtypes.float8_e4m3
NP_BF16 = ml_dtypes.bfloat16


def pack_rows(a2d: np.ndarray) -> np.ndarray:
    """[n*128, F] -> [128, n*F]: row r=k*128+p lands at partition p, block k.
    Makes each SBUF partition's data contiguous in DRAM (one fat DMA
    descriptor per partition instead of one per 128-row block)."""
    n = a2d.shape[0] // 128
    return np.ascontiguousarray(
        a2d.reshape(n, 128, a2d.shape[1]).transpose(1, 0, 2).reshape(128, -1)
    )


def build_graph() -> bass.Bass:
    mt_q = MT // XO_Q
    nc = bacc.Bacc(None, target_bir_lowering=False, debug=False, num_devices=8)
    at_ext = nc.declare_dram_parameter("at", [128, KT * NJ], FP8, isOutput=False)
    xt_ext = nc.declare_dram_parameter("xt", [128, KT * MB], FP8, isOutput=False)
    xo_ext = nc.declare_dram_parameter("xo", [128, MT * E], BF16, isOutput=False)
    out_ext = nc.declare_dram_parameter("out", [MB, NJ], BF16, isOutput=True)

    with tile.TileContext(nc) as tc:
        with (
            tc.tile_pool(name="big", bufs=1) as big,
            tc.tile_pool(name="k2p", bufs=K2) as k2p,
            tc.tile_pool(name="xop", bufs=XO_Q) as xop,
            tc.tile_pool(name="work", bufs=3) as work,
            tc.tile_pool(name="outs", bufs=3) as outs,
            tc.tile_pool(name="psum", bufs=2, space=bass.MemorySpace.PSUM) as psp,
        ):
            # PE warm-up: ~7us of dummy matmuls spanning the input-DMA head
            # so HAM un-throttles before the first real matmul arrives.
            neg_half = big.tile([128, 128], BF16)
            nc.vector.memset(neg_half, -0.5)
            warm_src = big.tile([128, 512], BF16)
            nc.vector.memset(warm_src, 0.125)
            warm_ps = psp.tile([128, PSW], F32, tag="ps", name="warm_ps")
            for wi in range(24):
                nc.tensor.matmul(
                    warm_ps[:64, :512], neg_half[:, :64], warm_src,
                    start=(wi == 0), stop=(wi == 23),
                )

            # Single fat loads: per-dma_start DMA rate scales with size
            # (>1MB ~ 341GB/s), so one dma_start per input beats k-sliced.
            at_s = big.tile([128, KT, NJ], FP8)
            nc.sync.dma_start(
                out=at_s, in_=at_ext[:].rearrange("p (k n) -> p k n", k=KT)
            )
            xo_r = xo_ext[:].rearrange("p (q e) -> p q e", q=XO_Q)
            xo_s = []
            o_t = xop.tile([128, mt_q * E], BF16, tag="xo", name="xo0")
            nc.sync.dma_start(out=o_t, in_=xo_r[:, 0, :])
            xo_s.append(o_t)
            xt_s = big.tile([128, KT, MB], FP8)
            nc.sync.dma_start(
                out=xt_s, in_=xt_ext[:].rearrange("p (k b) -> p k b", k=KT)
            )
            for q in range(1, XO_Q):
                o_t = xop.tile([128, mt_q * E], BF16, tag="xo", name=f"xo{q}")
                nc.sync.dma_start(out=o_t, in_=xo_r[:, q, :])
                xo_s.append(o_t)

            # sq_at = at*at in bf16 (feeds the a2 broadcast matmul).
            sq2 = []
            for q in range(K2):
                s_t = k2p.tile([128, 2, NJ], BF16, tag="sqat", name=f"sqat{q}")
                nc.vector.tensor_mul(s_t, at_s[:, 2 * q : 2 * q + 2, :], at_s[:, 2 * q : 2 * q + 2, :])
                sq2.append(s_t)

            a2b = big.tile([128, NJ], F32)  # -0.5*a2[j], same on every partition

            def emit_a2_setup():
                ps = psp.tile([128, PSW], F32, tag="ps", name="psa2")
                for n0, w in N_CHUNKS:
                    for k in range(KT):
                        nc.tensor.matmul(
                            ps[:, n0 : n0 + w],
                            neg_half,
                            sq2[k // 2][:, k % 2, n0 : n0 + w],
                            start=(k == 0), stop=(k == KT - 1),
                        )
                nc.scalar.copy(a2b, ps[:, :NJ])

            for m in range(MT):
                pts = psp.tile([128, PSW], F32, tag="ps", name=f"ps{m}")
                for q in range(K2):
                    lhsT = xt_s[:, 2 * q : 2 * q + 2, ts(m, 128)]
                    for n0, w in N_CHUNKS:
                        nc.tensor.matmul(
                            pts[:, n0 : n0 + w],
                            lhsT,
                            at_s[:, 2 * q : 2 * q + 2, n0 : n0 + w],
                            start=(q == 0), stop=(q == K2 - 1),
                            perf_mode=mybir.MatmulPerfMode.DoubleRow,
                        )
                if m == 0:
                    # Traced after m0's matmuls: PE reaches these once at has
                    # landed; the result is ready for m0's epilogue.
                    emit_a2_setup()

                sq_x = work.tile([128, E], BF16, tag="sqx")
                x2 = work.tile([128, 1], F32, tag="x2")
                sq_r = work.tile([128, E], BF16, tag="sqr")
                xo_m = xo_s[m // mt_q][:, (m % mt_q) * E : (m % mt_q + 1) * E]
                nc.gpsimd.tensor_mul(sq_x, xo_m, xo_m)
                nc.scalar.activation(
                    sq_r, sq_x,
                    mybir.ActivationFunctionType.Identity, accum_out=x2,
                )

                out_t = outs.tile([128, NJ], BF16, tag="out", name=f"out{m}")
                halves = [(0, NJ)] if m < MT - 2 else [(0, NJ // 2), (NJ // 2, NJ)]
                for h0, h1 in halves:
                    t = work.tile([128, NJ], F32, tag="t", name=f"t{m}_{h0}")
                    nc.vector.tensor_add(
                        t[:, : h1 - h0], pts[:, h0:h1], a2b[:, h0:h1]
                    )
                    nc.scalar.activation(
                        out_t[:, h0:h1], t[:, : h1 - h0],
                        mybir.ActivationFunctionType.Sqrt,
                        bias=x2, scale=-2.0,
                    )
                    nc.sync.dma_start(
                        out=out_ext[ts(m, 128), h0:h1], in_=out_t[:, h0:h1]
                    )

    nc.compile()
    return nc


def make_in_maps(x32: np.ndarray, a32: np.ndarray) -> list[dict[str, np.ndarray]]:
    xt_f8 = x32.T.astype(NP_FP8)           # [E, B]
    xo_bf = x32.astype(NP_BF16)            # [B, E]
    at_f8 = a32.T.astype(NP_FP8)           # [E, J]
    in_maps = []
    for c in range(8):
        g, h = c // RJ, c % RJ
        in_maps.append({
            "at": pack_rows(at_f8[:, h * NJ : (h + 1) * NJ]),
            "xt": pack_rows(xt_f8[:, g * MB : (g + 1) * MB]),
            "xo": pack_rows(xo_bf[g * MB : (g + 1) * MB, :]),
        })
    return in_maps


def kernel(x: np.ndarray, anchors: np.ndarray) -> np.ndarray:
    x32 = np.asarray(x, dtype=np.float32)
    a32 = np.asarray(anchors, dtype=np.float32).reshape(J, E)

    nc = build_graph()
    in_maps = make_in_maps(x32, a32)
    results = run_bass_kernel_spmd(nc, in_maps, core_ids=list(range(8))).results

    out = np.empty((B, J), dtype=np.float32)
    for c in range(8):
        g, h = c // RJ, c % RJ
        out[g * MB : (g + 1) * MB, h * NJ : (h + 1) * NJ] = results[c][
            "out"
        ].astype(np.float32)
    return out.reshape(B, C, A)


# revision 18
# speedup vs baseline: 1.2266x; 1.2266x over previous
"""Pairwise L2 distance kernel: x [4096,768], anchors [100,64,768] -> [4096,100,64].

Distributed over 8 TRN2 NeuronCores as a 2x4 grid: batch (4096) split in 2,
anchor index (6400) split in 4.  Each core computes a [2048,1600] output block
as sqrt(x2[b] + a2[j] - 2*x@A^T).

The x@A^T matmul runs in fp8e4m3 with DoubleRow (K=256 per pass, fp32 PSUM
accumulate); norms are computed on device (x2 from a bf16 copy of x via
ACT Square+accum, a2 via DVE square + all-(-0.5) ones-matmul broadcast).
Host does layout transforms only (transpose, dtype cast, partition packing).
"""

import sys

import numpy as np

for _p in ("/opt/trn_rl_repo", "/root/.axon_site/_ro/trn_rl_repo"):
    if _p not in sys.path:
        sys.path.append(_p)

import ml_dtypes

import concourse.bass as bass
import concourse.tile as tile
from concourse import bacc, mybir
from concourse.bass import ts
from concourse.bass_utils import run_bass_kernel_spmd

B, C, A, E = 4096, 100, 64, 768
J = C * A                 # 6400 flattened anchors
RB, RJ = 2, 4             # batch groups x anchor groups = 8 cores
MB = B // RB              # 2048 batch rows per core
NJ = J // RJ              # 1600 anchor cols per core
KT = E // 128             # 6 contraction tiles of 128
K2 = KT // 2              # 3 DoubleRow k-pair passes
MT = MB // 128            # 16 m-tiles per core
XO_Q = 8                  # xo arrives in 8 DMA slices
N_CHUNKS = [(0, 512), (512, 512), (1024, 512), (1536, 64)]
PSW = 2048                # psum tile width (4 banks), holds all chunks

FP8 = mybir.dt.float8e4
BF16 = mybir.dt.bfloat16
F32 = mybir.dt.float32
NP_FP8 = ml_dtypes.float8_e4m3
NP_BF16 = ml_dtypes.bfloat16


def pack_rows(a2d: np.ndarray) -> np.ndarray:
    """[n*128, F] -> [128, n*F]: row r=k*128+p lands at partition p, block k.
    Makes each SBUF partition's data contiguous in DRAM (one fat DMA
    descriptor per partition instead of one per 128-row block)."""
    n = a2d.shape[0] // 128
    return np.ascontiguousarray(
        a2d.reshape(n, 128, a2d.shape[1]).transpose(1, 0, 2).reshape(128, -1)
    )


def build_graph() -> bass.Bass:
    mt_q = MT // XO_Q
    nc = bacc.Bacc(None, target_bir_lowering=False, debug=False, num_devices=8)
    at_ext = nc.declare_dram_parameter("at", [128, KT * NJ], FP8, isOutput=False)
    xt_ext = nc.declare_dram_parameter("xt", [128, KT * MB], FP8, isOutput=False)
    xo_ext = nc.declare_dram_parameter("xo", [128, MT * E], BF16, isOutput=False)
    out_ext = nc.declare_dram_parameter("out", [MB, NJ], BF16, isOutput=True)

    with tile.TileContext(nc) as tc:
        with (
            tc.tile_pool(name="big", bufs=1) as big,
            tc.tile_pool(name="k2p", bufs=K2) as k2p,
            tc.tile_pool(name="xop", bufs=XO_Q) as xop,
            tc.tile_pool(name="work", bufs=3) as work,
            tc.tile_pool(name="outs", bufs=3) as outs,
            tc.tile_pool(name="psum", bufs=2, space=bass.MemorySpace.PSUM) as psp,
        ):
            # PE warm-up: ~7us of dummy matmuls spanning the input-DMA head
            # so HAM un-throttles before the first real matmul arrives.
            neg_half = big.tile([128, 128], BF16)
            nc.vector.memset(neg_half, -0.5)
            warm_src = big.tile([128, 512], BF16)
            nc.vector.memset(warm_src, 0.125)
            warm_ps = psp.tile([128, PSW], F32, tag="ps", name="warm_ps")
            for wi in range(24):
                nc.tensor.matmul(
                    warm_ps[:64, :512], neg_half[:, :64], warm_src,
                    start=(wi == 0), stop=(wi == 23),
                )

            # Single fat loads: per-dma_start DMA rate scales with size
            # (>1MB ~ 341GB/s), so one dma_start per input beats k-sliced.
            at_s = big.tile([128, KT, NJ], FP8)
            nc.sync.dma_start(
                out=at_s, in_=at_ext[:].rearrange("p (k n) -> p k n", k=KT)
            )
            xo_r = xo_ext[:].rearrange("p (q e) -> p q e", q=XO_Q)
            xo_s = []
            o_t = xop.tile([128, mt_q * E], BF16, tag="xo", name="xo0")
            nc.sync.dma_start(out=o_t, in_=xo_r[:, 0, :])
            xo_s.append(o_t)
            xt_s = big.tile([128, KT, MB], FP8)
            nc.sync.dma_start(
                out=xt_s, in_=xt_ext[:].rearrange("p (k b) -> p k b", k=KT)
            )
            for q in range(1, XO_Q):
                o_t = xop.tile([128, mt_q * E], BF16, tag="xo", name=f"xo{q}")
                nc.sync.dma_start(out=o_t, in_=xo_r[:, q, :])
                xo_s.append(o_t)

            # sq_at = at*at in bf16 (feeds the a2 broadcast matmul).
            sq2 = []
            for q in range(K2):
                s_t = k2p.tile([128, 2, NJ], BF16, tag="sqat", name=f"sqat{q}")
                nc.vector.tensor_mul(s_t, at_s[:, 2 * q : 2 * q + 2, :], at_s[:, 2 * q : 2 * q + 2, :])
                sq2.append(s_t)

            a2b = big.tile([128, NJ], F32)  # -0.5*a2[j], same on every partition

            def emit_a2_setup():
                ps = psp.tile([128, PSW], F32, tag="ps", name="psa2")
                for n0, w in N_CHUNKS:
                    for k in range(KT):
                        nc.tensor.matmul(
                            ps[:, n0 : n0 + w],
                            neg_half,
                            sq2[k // 2][:, k % 2, n0 : n0 + w],
                            start=(k == 0), stop=(k == KT - 1),
                        )
                nc.scalar.copy(a2b, ps[:, :NJ])

            for m in range(MT):
                pts = psp.tile([128, PSW], F32, tag="ps", name=f"ps{m}")
                for q in range(K2):
                    lhsT = xt_s[:, 2 * q : 2 * q + 2, ts(m, 128)]
                    for n0, w in N_CHUNKS:
                        nc.tensor.matmul(
                            pts[:, n0 : n0 + w],
                            lhsT,
                            at_s[:, 2 * q : 2 * q + 2, n0 : n0 + w],
                            start=(q == 0), stop=(q == K2 - 1),
                            perf_mode=mybir.MatmulPerfMode.DoubleRow,
                        )
                if m == 0:
                    # Traced after m0's matmuls: PE reaches these once at has
                    # landed; the result is ready for m0's epilogue.
                    emit_a2_setup()

                sq_x = work.tile([128, E], BF16, tag="sqx")
                x2 = work.tile([128, 1], F32, tag="x2")
                nc.scalar.activation(
                    sq_x, xo_s[m // mt_q][:, (m % mt_q) * E : (m % mt_q + 1) * E],
                    mybir.ActivationFunctionType.Square, accum_out=x2,
                )

                out_t = outs.tile([128, NJ], BF16, tag="out", name=f"out{m}")
                halves = [(0, NJ)] if m < MT - 2 else [(0, NJ // 2), (NJ // 2, NJ)]
                for h0, h1 in halves:
                    t = work.tile([128, NJ], F32, tag="t", name=f"t{m}_{h0}")
                    nc.vector.tensor_add(
                        t[:, : h1 - h0], pts[:, h0:h1], a2b[:, h0:h1]
                    )
                    nc.scalar.activation(
                        out_t[:, h0:h1], t[:, : h1 - h0],
                        mybir.ActivationFunctionType.Sqrt,
                        bias=x2, scale=-2.0,
                    )
                    nc.sync.dma_start(
                        out=out_ext[ts(m, 128), h0:h1], in_=out_t[:, h0:h1]
                    )

    nc.compile()
    return nc


def make_in_maps(x32: np.ndarray, a32: np.ndarray) -> list[dict[str, np.ndarray]]:
    xt_f8 = x32.T.astype(NP_FP8)           # [E, B]
    xo_bf = x32.astype(NP_BF16)            # [B, E]
    at_f8 = a32.T.astype(NP_FP8)           # [E, J]
    in_maps = []
    for c in range(8):
        g, h = c // RJ, c % RJ
        in_maps.append({
            "at": pack_rows(at_f8[:, h * NJ : (h + 1) * NJ]),
            "xt": pack_rows(xt_f8[:, g * MB : (g + 1) * MB]),
            "xo": pack_rows(xo_bf[g * MB : (g + 1) * MB, :]),
        })
    return in_maps


def kernel(x: np.ndarray, anchors: np.ndarray) -> np.ndarray:
    x32 = np.asarray(x, dtype=np.float32)
    a32 = np.asarray(anchors, dtype=np.float32).reshape(J, E)

    nc = build_graph()
    in_maps = make_in_maps(x32, a32)
    results = run_bass_kernel_spmd(nc, in_maps, core_ids=list(range(8))).results

    out = np.empty((B, J), dtype=np.float32)
    for c in range(8):
        g, h = c // RJ, c % RJ
        out[g * MB : (g + 1) * MB, h * NJ : (h + 1) * NJ] = results[c][
            "out"
        ].astype(np.float32)
    return out.reshape(B, C, A)


# revision 19
# speedup vs baseline: 1.2470x; 1.0166x over previous
"""Pairwise L2 distance kernel: x [4096,768], anchors [100,64,768] -> [4096,100,64].

Distributed over 8 TRN2 NeuronCores as a 2x4 grid: batch (4096) split in 2,
anchor index (6400) split in 4.  Each core computes a [2048,1600] output block
as sqrt(x2[b] + a2[j] - 2*x@A^T).

The x@A^T matmul runs in fp8e4m3 with DoubleRow (K=256 per pass, fp32 PSUM
accumulate); norms are computed on device (x2 from a bf16 copy of x via
ACT Square+accum, a2 via DVE square + all-(-0.5) ones-matmul broadcast).
Host does layout transforms only (transpose, dtype cast, partition packing).
"""

import sys

import numpy as np

for _p in ("/opt/trn_rl_repo", "/root/.axon_site/_ro/trn_rl_repo"):
    if _p not in sys.path:
        sys.path.append(_p)

import ml_dtypes

import concourse.bass as bass
import concourse.tile as tile
from concourse import bacc, mybir
from concourse.bass import ts
from concourse.bass_utils import run_bass_kernel_spmd

B, C, A, E = 4096, 100, 64, 768
J = C * A                 # 6400 flattened anchors
RB, RJ = 2, 4             # batch groups x anchor groups = 8 cores
MB = B // RB              # 2048 batch rows per core
NJ = J // RJ              # 1600 anchor cols per core
KT = E // 128             # 6 contraction tiles of 128
K2 = KT // 2              # 3 DoubleRow k-pair passes
MT = MB // 128            # 16 m-tiles per core
XO_Q = 8                  # xo arrives in 8 DMA slices
N_CHUNKS = [(0, 512), (512, 512), (1024, 512), (1536, 64)]
PSW = 2048                # psum tile width (4 banks), holds all chunks

FP8 = mybir.dt.float8e4
BF16 = mybir.dt.bfloat16
F32 = mybir.dt.float32
NP_FP8 = ml_dtypes.float8_e4m3
NP_BF16 = ml_dtypes.bfloat16


def pack_rows(a2d: np.ndarray) -> np.ndarray:
    """[n*128, F] -> [128, n*F]: row r=k*128+p lands at partition p, block k.
    Makes each SBUF partition's data contiguous in DRAM (one fat DMA
    descriptor per partition instead of one per 128-row block)."""
    n = a2d.shape[0] // 128
    return np.ascontiguousarray(
        a2d.reshape(n, 128, a2d.shape[1]).transpose(1, 0, 2).reshape(128, -1)
    )


def build_graph() -> bass.Bass:
    mt_q = MT // XO_Q
    nc = bacc.Bacc(None, target_bir_lowering=False, debug=False, num_devices=8)
    at_ext = nc.declare_dram_parameter("at", [128, KT * NJ], FP8, isOutput=False)
    xt_ext = nc.declare_dram_parameter("xt", [128, KT * MB], FP8, isOutput=False)
    xo_ext = nc.declare_dram_parameter("xo", [128, MT * E], BF16, isOutput=False)
    out_ext = nc.declare_dram_parameter("out", [MB, NJ], BF16, isOutput=True)

    with tile.TileContext(nc) as tc:
        with (
            tc.tile_pool(name="big", bufs=1) as big,
            tc.tile_pool(name="k2p", bufs=K2) as k2p,
            tc.tile_pool(name="xop", bufs=XO_Q) as xop,
            tc.tile_pool(name="work", bufs=5) as work,
            tc.tile_pool(name="outs", bufs=4) as outs,
            tc.tile_pool(name="psum", bufs=2, space=bass.MemorySpace.PSUM) as psp,
        ):
            # PE warm-up: ~7us of dummy matmuls spanning the input-DMA head
            # so HAM un-throttles before the first real matmul arrives.
            neg_half = big.tile([128, 128], BF16)
            nc.vector.memset(neg_half, -0.5)
            warm_src = big.tile([128, 512], BF16)
            nc.vector.memset(warm_src, 0.125)
            warm_ps = psp.tile([128, PSW], F32, tag="ps", name="warm_ps")
            for wi in range(24):
                nc.tensor.matmul(
                    warm_ps[:64, :512], neg_half[:, :64], warm_src,
                    start=(wi == 0), stop=(wi == 23),
                )

            # Single fat loads: per-dma_start DMA rate scales with size
            # (>1MB ~ 341GB/s), so one dma_start per input beats k-sliced.
            at_s = big.tile([128, KT, NJ], FP8)
            nc.sync.dma_start(
                out=at_s, in_=at_ext[:].rearrange("p (k n) -> p k n", k=KT)
            )
            xo_r = xo_ext[:].rearrange("p (q e) -> p q e", q=XO_Q)
            xo_s = []
            o_t = xop.tile([128, mt_q * E], BF16, tag="xo", name="xo0")
            nc.sync.dma_start(out=o_t, in_=xo_r[:, 0, :])
            xo_s.append(o_t)
            xt_s = big.tile([128, KT, MB], FP8)
            nc.sync.dma_start(
                out=xt_s, in_=xt_ext[:].rearrange("p (k b) -> p k b", k=KT)
            )
            for q in range(1, XO_Q):
                o_t = xop.tile([128, mt_q * E], BF16, tag="xo", name=f"xo{q}")
                nc.sync.dma_start(out=o_t, in_=xo_r[:, q, :])
                xo_s.append(o_t)

            # sq_at = at*at in bf16 (feeds the a2 broadcast matmul).
            sq2 = []
            for q in range(K2):
                s_t = k2p.tile([128, 2, NJ], BF16, tag="sqat", name=f"sqat{q}")
                a_sl = at_s[:, 2 * q : 2 * q + 2, :]
                if q == 1:
                    nc.scalar.activation(
                        s_t, a_sl, mybir.ActivationFunctionType.Square
                    )
                else:
                    nc.vector.tensor_mul(s_t, a_sl, a_sl)
                sq2.append(s_t)

            a2b = big.tile([128, NJ], F32)  # -0.5*a2[j], same on every partition

            def emit_a2_setup():
                ps = psp.tile([128, PSW], F32, tag="ps", name="psa2")
                for n0, w in N_CHUNKS:
                    for k in range(KT):
                        nc.tensor.matmul(
                            ps[:, n0 : n0 + w],
                            neg_half,
                            sq2[k // 2][:, k % 2, n0 : n0 + w],
                            start=(k == 0), stop=(k == KT - 1),
                        )
                nc.scalar.copy(a2b, ps[:, :NJ])

            for m in range(MT):
                pts = psp.tile([128, PSW], F32, tag="ps", name=f"ps{m}")
                for q in range(K2):
                    lhsT = xt_s[:, 2 * q : 2 * q + 2, ts(m, 128)]
                    for n0, w in N_CHUNKS:
                        nc.tensor.matmul(
                            pts[:, n0 : n0 + w],
                            lhsT,
                            at_s[:, 2 * q : 2 * q + 2, n0 : n0 + w],
                            start=(q == 0), stop=(q == K2 - 1),
                            perf_mode=mybir.MatmulPerfMode.DoubleRow,
                        )
                if m == 0:
                    # Traced after m0's matmuls: PE reaches these once at has
                    # landed; the result is ready for m0's epilogue.
                    emit_a2_setup()

                sq_x = work.tile([128, E], BF16, tag="sqx")
                x2 = work.tile([128, 1], F32, tag="x2")
                nc.scalar.activation(
                    sq_x, xo_s[m // mt_q][:, (m % mt_q) * E : (m % mt_q + 1) * E],
                    mybir.ActivationFunctionType.Square, accum_out=x2,
                )

                out_t = outs.tile([128, NJ], BF16, tag="out", name=f"out{m}")
                halves = [(0, NJ)] if m < MT - 2 else [(0, NJ // 2), (NJ // 2, NJ)]
                for h0, h1 in halves:
                    t = work.tile([128, NJ], F32, tag="t", name=f"t{m}_{h0}")
                    if m < 3:
                        # a2b isn't ready yet early on: free the PSUM slot
                        # with an ACT copy, add a2b in place later on DVE.
                        nc.scalar.copy(t[:, : h1 - h0], pts[:, h0:h1])
                        nc.vector.tensor_add(
                            t[:, : h1 - h0], t[:, : h1 - h0], a2b[:, h0:h1]
                        )
                    else:
                        nc.vector.tensor_add(
                            t[:, : h1 - h0], pts[:, h0:h1], a2b[:, h0:h1]
                        )
                    nc.scalar.activation(
                        out_t[:, h0:h1], t[:, : h1 - h0],
                        mybir.ActivationFunctionType.Sqrt,
                        bias=x2, scale=-2.0,
                    )
                    nc.sync.dma_start(
                        out=out_ext[ts(m, 128), h0:h1], in_=out_t[:, h0:h1]
                    )

    nc.compile()
    return nc


def make_in_maps(x32: np.ndarray, a32: np.ndarray) -> list[dict[str, np.ndarray]]:
    xt_f8 = x32.T.astype(NP_FP8)           # [E, B]
    xo_bf = x32.astype(NP_BF16)            # [B, E]
    at_f8 = a32.T.astype(NP_FP8)           # [E, J]
    in_maps = []
    for c in range(8):
        g, h = c // RJ, c % RJ
        in_maps.append({
            "at": pack_rows(at_f8[:, h * NJ : (h + 1) * NJ]),
            "xt": pack_rows(xt_f8[:, g * MB : (g + 1) * MB]),
            "xo": pack_rows(xo_bf[g * MB : (g + 1) * MB, :]),
        })
    return in_maps


def kernel(x: np.ndarray, anchors: np.ndarray) -> np.ndarray:
    x32 = np.asarray(x, dtype=np.float32)
    a32 = np.asarray(anchors, dtype=np.float32).reshape(J, E)

    nc = build_graph()
    in_maps = make_in_maps(x32, a32)
    results = run_bass_kernel_spmd(nc, in_maps, core_ids=list(range(8))).results

    out = np.empty((B, J), dtype=np.float32)
    for c in range(8):
        g, h = c // RJ, c % RJ
        out[g * MB : (g + 1) * MB, h * NJ : (h + 1) * NJ] = results[c][
            "out"
        ].astype(np.float32)
    return out.reshape(B, C, A)


# revision 21
# speedup vs baseline: 1.2777x; 1.0246x over previous
"""Pairwise L2 distance kernel: x [4096,768], anchors [100,64,768] -> [4096,100,64].

Distributed over 8 TRN2 NeuronCores as a 2x4 grid: batch (4096) split in 2,
anchor index (6400) split in 4.  Each core computes a [2048,1600] output block
as sqrt(x2[b] + a2[j] - 2*x@A^T).

The x@A^T matmul runs in fp8e4m3 with DoubleRow (K=256 per pass, fp32 PSUM
accumulate); norms are computed on device (x2 from a bf16 copy of x via
ACT Square+accum, a2 via DVE square + all-(-0.5) ones-matmul broadcast).
Host does layout transforms only (transpose, dtype cast, partition packing).
"""

import sys

import numpy as np

for _p in ("/opt/trn_rl_repo", "/root/.axon_site/_ro/trn_rl_repo"):
    if _p not in sys.path:
        sys.path.append(_p)

import ml_dtypes

import concourse.bass as bass
import concourse.tile as tile
from concourse import bacc, mybir
from concourse.bass import ts
from concourse.bass_utils import run_bass_kernel_spmd

B, C, A, E = 4096, 100, 64, 768
J = C * A                 # 6400 flattened anchors
RB, RJ = 2, 4             # batch groups x anchor groups = 8 cores
MB = B // RB              # 2048 batch rows per core
NJ = J // RJ              # 1600 anchor cols per core
KT = E // 128             # 6 contraction tiles of 128
K2 = KT // 2              # 3 DoubleRow k-pair passes
MT = MB // 128            # 16 m-tiles per core
XO_Q = 8                  # xo arrives in 8 DMA slices
N_CHUNKS = [(0, 512), (512, 512), (1024, 512), (1536, 64)]
PSW = 2048                # psum tile width (4 banks), holds all chunks

FP8 = mybir.dt.float8e4
BF16 = mybir.dt.bfloat16
F32 = mybir.dt.float32
NP_FP8 = ml_dtypes.float8_e4m3
NP_BF16 = ml_dtypes.bfloat16


def pack_rows(a2d: np.ndarray) -> np.ndarray:
    """[n*128, F] -> [128, n*F]: row r=k*128+p lands at partition p, block k.
    Makes each SBUF partition's data contiguous in DRAM (one fat DMA
    descriptor per partition instead of one per 128-row block)."""
    n = a2d.shape[0] // 128
    return np.ascontiguousarray(
        a2d.reshape(n, 128, a2d.shape[1]).transpose(1, 0, 2).reshape(128, -1)
    )


def build_graph() -> bass.Bass:
    mt_q = MT // XO_Q
    nc = bacc.Bacc(None, target_bir_lowering=False, debug=False, num_devices=8)
    at_ext = nc.declare_dram_parameter("at", [128, KT * NJ], FP8, isOutput=False)
    xt_ext = nc.declare_dram_parameter("xt", [128, KT * MB], FP8, isOutput=False)
    xo_ext = nc.declare_dram_parameter("xo", [128, MT * E], BF16, isOutput=False)
    out_ext = nc.declare_dram_parameter("out", [MB, NJ], BF16, isOutput=True)

    with tile.TileContext(nc) as tc:
        with (
            tc.tile_pool(name="big", bufs=1) as big,
            tc.tile_pool(name="k2p", bufs=K2) as k2p,
            tc.tile_pool(name="xop", bufs=XO_Q) as xop,
            tc.tile_pool(name="work", bufs=5) as work,
            tc.tile_pool(name="outs", bufs=4) as outs,
            tc.tile_pool(name="psum", bufs=2, space=bass.MemorySpace.PSUM) as psp,
        ):
            # PE warm-up: ~7us of dummy matmuls spanning the input-DMA head
            # so HAM un-throttles before the first real matmul arrives.
            neg_half = big.tile([128, 128], BF16)
            nc.vector.memset(neg_half, -0.5)
            warm_src = big.tile([128, 512], BF16)
            nc.vector.memset(warm_src, 0.125)
            warm_ps = psp.tile([128, PSW], F32, tag="ps", name="warm_ps")
            for wi in range(24):
                nc.tensor.matmul(
                    warm_ps[:64, :512], neg_half[:, :64], warm_src,
                    start=(wi == 0), stop=(wi == 23),
                )

            # Single fat loads: per-dma_start DMA rate scales with size
            # (>1MB ~ 341GB/s), so one dma_start per input beats k-sliced.
            at_s = big.tile([128, KT, NJ], FP8)
            nc.sync.dma_start(
                out=at_s, in_=at_ext[:].rearrange("p (k n) -> p k n", k=KT)
            )
            xo_r = xo_ext[:].rearrange("p (q e) -> p q e", q=XO_Q)
            xo_s = []
            o_t = xop.tile([128, mt_q * E], BF16, tag="xo", name="xo0")
            nc.sync.dma_start(out=o_t, in_=xo_r[:, 0, :])
            xo_s.append(o_t)
            xt_s = big.tile([128, KT, MB], FP8)
            nc.sync.dma_start(
                out=xt_s, in_=xt_ext[:].rearrange("p (k b) -> p k b", k=KT)
            )
            for q in range(1, XO_Q):
                o_t = xop.tile([128, mt_q * E], BF16, tag="xo", name=f"xo{q}")
                nc.sync.dma_start(out=o_t, in_=xo_r[:, q, :])
                xo_s.append(o_t)

            # sq_at = at*at in bf16 (feeds the a2 broadcast matmul).
            sq2 = []
            for q in range(K2):
                s_t = k2p.tile([128, 2, NJ], BF16, tag="sqat", name=f"sqat{q}")
                a_sl = at_s[:, 2 * q : 2 * q + 2, :]
                if q == 1:
                    nc.scalar.activation(
                        s_t, a_sl, mybir.ActivationFunctionType.Square
                    )
                else:
                    nc.vector.tensor_mul(s_t, a_sl, a_sl)
                sq2.append(s_t)

            a2b = big.tile([128, NJ], F32)  # -0.5*a2[j], same on every partition

            def emit_a2_setup():
                ps = psp.tile([128, PSW], F32, tag="ps", name="psa2")
                for n0, w in N_CHUNKS:
                    for k in range(KT):
                        nc.tensor.matmul(
                            ps[:, n0 : n0 + w],
                            neg_half,
                            sq2[k // 2][:, k % 2, n0 : n0 + w],
                            start=(k == 0), stop=(k == KT - 1),
                        )
                nc.scalar.copy(a2b, ps[:, :NJ])

            for m in range(MT):
                pts = psp.tile([128, PSW], F32, tag="ps", name=f"ps{m}")
                for q in range(K2):
                    lhsT = xt_s[:, 2 * q : 2 * q + 2, ts(m, 128)]
                    for n0, w in N_CHUNKS:
                        nc.tensor.matmul(
                            pts[:, n0 : n0 + w],
                            lhsT,
                            at_s[:, 2 * q : 2 * q + 2, n0 : n0 + w],
                            start=(q == 0), stop=(q == K2 - 1),
                            perf_mode=mybir.MatmulPerfMode.DoubleRow,
                        )
                if m == 0:
                    # Traced after m0's matmuls: PE reaches these once at has
                    # landed; the result is ready for m0's epilogue.
                    emit_a2_setup()

                sq_x = work.tile([128, E], BF16, tag="sqx")
                x2 = work.tile([128, 1], F32, tag="x2")
                nc.scalar.activation(
                    sq_x, xo_s[m // mt_q][:, (m % mt_q) * E : (m % mt_q + 1) * E],
                    mybir.ActivationFunctionType.Square, accum_out=x2,
                )

                out_t = outs.tile([128, NJ], BF16, tag="out", name=f"out{m}")
                halves = [(0, NJ)] if m < MT - 2 else [(0, NJ // 2), (NJ // 2, NJ)]
                for h0, h1 in halves:
                    t = work.tile([128, NJ], F32, tag="t", name=f"t{m}_{h0}")
                    if m < 3:
                        # a2b isn't ready yet early on: free the PSUM slot
                        # with an ACT copy, add a2b in place later on DVE.
                        nc.scalar.copy(t[:, : h1 - h0], pts[:, h0:h1])
                        nc.vector.tensor_add(
                            t[:, : h1 - h0], t[:, : h1 - h0], a2b[:, h0:h1]
                        )
                    else:
                        nc.vector.tensor_add(
                            t[:, : h1 - h0], pts[:, h0:h1], a2b[:, h0:h1]
                        )
                    nc.scalar.activation(
                        out_t[:, h0:h1], t[:, : h1 - h0],
                        mybir.ActivationFunctionType.Sqrt,
                        bias=x2, scale=-2.0,
                    )
                    nc.sync.dma_start(
                        out=out_ext[ts(m, 128), h0:h1], in_=out_t[:, h0:h1]
                    )

    nc.compile()
    return nc


def make_in_maps(x32: np.ndarray, a32: np.ndarray) -> list[dict[str, np.ndarray]]:
    xt_f8 = x32.T.astype(NP_FP8)           # [E, B]
    xo_bf = x32.astype(NP_BF16)            # [B, E]
    at_f8 = a32.T.astype(NP_FP8)           # [E, J]
    in_maps = []
    for c in range(8):
        g, h = c // RJ, c % RJ
        in_maps.append({
            "at": pack_rows(at_f8[:, h * NJ : (h + 1) * NJ]),
            "xt": pack_rows(xt_f8[:, g * MB : (g + 1) * MB]),
            "xo": pack_rows(xo_bf[g * MB : (g + 1) * MB, :]),
        })
    return in_maps


def kernel(x: np.ndarray, anchors: np.ndarray) -> np.ndarray:
    x32 = np.asarray(x, dtype=np.float32)
    a32 = np.asarray(anchors, dtype=np.float32).reshape(J, E)

    nc = build_graph()
    in_maps = make_in_maps(x32, a32)
    results = run_bass_kernel_spmd(nc, in_maps, core_ids=list(range(8))).results

    out = np.empty((B, J), dtype=np.float32)
    for c in range(8):
        g, h = c // RJ, c % RJ
        out[g * MB : (g + 1) * MB, h * NJ : (h + 1) * NJ] = results[c][
            "out"
        ].astype(np.float32)
    return out.reshape(B, C, A)
